# revision 12
# baseline (speedup 1.0000x reference)
"""BitNet linear layer (b1.58-style) on 8 Trainium2 NeuronCores.

Computes: scale = 1e-4 + mean(|W|); q = clip(round(W/scale), -1, 1);
          out = scale * (x @ q.T)
for x [4, 2048, 2048] f32 and W [8192, 2048] f32.

Sharding: tensor-parallel over out_features. Each core gets the full x
(replicated) and a 1024-row shard of W; host concatenates the 8 per-core
[8192, 1024] outputs along the feature axis.

Per-core compute strategy (fp8 DoubleRow main loop):
  - The ternary q is exact in fp8e4. x is decomposed as
    hi = fp8(bf16(x)), lo = fp8(bf16(x) - hi). K-columns 0..NHILO*128-1
    are computed with (hi, lo) pairs (error ~bf16-level); the rest with
    hi only. NHILO=6 -> measured output rel err ~1.9e-2 < 2e-2 gate.
  - DoubleRow fp8 matmuls contract 256 SBUF rows per instruction at the
    same per-instruction cost as one 128-row bf16 matmul, so the main
    loop needs NSLOT=11 instructions per (m-tile, n-half) vs 16 bf16.
  - Layout: xq [128, NSLOT, 2, 128] per m-tile; j<NHILO -> (hi_j, lo_j),
    j>=NHILO -> (hi_a, hi_b) two k-tiles packed. qT mirrors it with q_j
    duplicated for hi/lo pairs.
  - x transposes ride the DMA XBAR (bf16 [128,2048] -> [128,16,128]
    block transpose, ~4.4us serialized, hidden under the main loop), so
    the Tensor engine runs only the DoubleRow stream + q transposes.
  - Startup: W + |W| AllReduce trigger first (with a junk warm-up
    AllReduce at t=0 to absorb CC ring setup); a shallow x prefix plus
    dummy identity matmuls keep the Tensor engine busy so HAM keeps the
    clock up; thresholds + q quantize follow the prefix in the DVE FIFO;
    the main loop stages x m-tiles LOOKAHEAD deep.
"""

import os
import sys

sys.path.insert(0, "/opt/trn_rl_repo")

import numpy as np

import concourse.bass as bass
import concourse.tile as tile
from concourse import bacc, mybir
from concourse.bass_utils import run_bass_kernel_spmd
from concourse.masks import make_identity
from concourse import bass_isa

F32 = mybir.dt.float32
BF16 = mybir.dt.bfloat16
FP8 = mybir.dt.float8e4

NCORES = 8
M = 8192          # tokens (4*2048)
K = 2048          # in_features
N_FULL = 8192     # out_features
NS = N_FULL // NCORES  # 1024 per-core shard
P = 128
KO = K // P       # 16 k-tiles
NO = NS // P      # 8 W-row tiles per shard
MT = M // P       # 64 m-tiles
W_ELEMS = float(N_FULL * K)  # 16777216, for the mean

NHILO = 6         # k-tiles 0..NHILO-1 get hi+lo pairs; rest hi only
NSLOT = NHILO + (KO - NHILO) // 2  # 11 DoubleRow slots
PREFIX = 10       # m-tiles of x pipeline emitted before quantize
LOOKAHEAD = 3     # m-tiles staged ahead of the main matmul loop
DUM_PER_MT = 16   # PE warm-up matmuls per prefix m-tile
DUM_TAIL = 200    # extra warm-up matmuls after the prefix

DR = mybir.MatmulPerfMode.DoubleRow
COPY = mybir.ActivationFunctionType.Copy


def build_nc():
    nc = bacc.Bacc("TRN2", target_bir_lowering=False, debug=False,
                   num_devices=NCORES)
    x_d = nc.dram_tensor("x", [M, K], F32, kind="ExternalInput")
    w_d = nc.dram_tensor("w", [NS, K], F32, kind="ExternalInput")
    o_d = nc.dram_tensor("out", [M, NS], F32, kind="ExternalOutput")
    x_ap, w_ap, o_ap = x_d.ap(), w_d.ap(), o_d.ap()

    with tile.TileContext(nc) as tc:
        with (
            tc.tile_pool(name="const", bufs=1) as const,
            tc.tile_pool(name="scal", bufs=1) as scal,
            tc.tile_pool(name="wpool", bufs=4) as wpool,
            tc.tile_pool(name="qtpool", bufs=2) as qtpool,
            tc.tile_pool(name="gpool", bufs=1) as gpool,
            tc.tile_pool(name="qT_pool", bufs=1) as qT_pool,
            tc.tile_pool(name="xpool", bufs=3) as xpool,
            tc.tile_pool(name="xbpool", bufs=3) as xbpool,
            tc.tile_pool(name="xTpool", bufs=5) as xTpool,
            tc.tile_pool(name="xqpool", bufs=PREFIX + LOOKAHEAD + 2) as xqpool,
            tc.tile_pool(name="opool", bufs=2) as opool,
            tc.tile_pool(name="psum_t", bufs=2, space="PSUM") as psum_t,
            tc.tile_pool(name="psum_o", bufs=4, space="PSUM") as psum_o,
            tc.tile_pool(name="psum_d", bufs=2, space="PSUM") as psum_d,
            tc.tile_pool(name="dram", bufs=1, space="DRAM") as dram,
        ):
            ident = const.tile([P, P], BF16, name="ident")
            make_identity(nc, ident)
            identq = const.tile([P, P], FP8, name="identq")
            make_identity(nc, identq)

            # ---- warm-up AllReduce on junk: absorbs CC ring setup -------
            ccw_in = dram.tile([1, 1], F32, name="ccw_in")
            ccw_out = dram.tile([1, 1], F32, name="ccw_out",
                                addr_space="Shared")
            nc.gpsimd.dma_start(ccw_in[:], ident[0:1, 0:1])
            nc.gpsimd.collective_compute(
                "AllReduce", mybir.AluOpType.add,
                replica_groups=[list(range(NCORES))],
                ins=[ccw_in[:].opt()], outs=[ccw_out[:].opt()])

            # ---- x pipeline stages --------------------------------------
            def x_load(mt):
                xt = xpool.tile([P, K], F32, name=f"x_{mt}", tag="x")
                nc.scalar.dma_start(xt[:], x_ap[mt * P:(mt + 1) * P, :])
                return xt

            def x_stage(mt, xt, xb_out=None):
                """cast -> XBAR DMA transpose -> quantize into xq slots.

                Slot layout (NHILO=6): transposed k-tile ko lives at
                xT[:, ko, :].
                  kos 0-5   -> hi xq[:,ko,0,:], lo xq[:,ko,1,:]
                  kos 6..15 -> hi only: xq[:,6,:,:] (6,7), xq[:,7:9,:,:]
                              (8-11, ACT), xq[:,9:11,:,:] (12-15, ACT)
                """
                xb = xbpool.tile([P, K], BF16, name=f"xb_{mt}", tag="xb")
                if xb_out is not None:
                    xb_out[0] = xb
                nc.vector.tensor_copy(xb[:], xt[:])
                xT = xTpool.tile([P, KO, P], BF16, name=f"xT_{mt}", tag="xT")
                nc.sync.dma_start(xT[:], xb[:], transpose=True)
                xq = xqpool.tile([P, NSLOT, 2, P], FP8, name=f"xq_{mt}",
                                 tag="xq")
                hi6 = xq[:, 0:NHILO, 0, :]
                nc.vector.tensor_copy(hi6, xT[:, 0:NHILO, :])
                nc.vector.tensor_tensor(
                    xq[:, 0:NHILO, 1, :], xT[:, 0:NHILO, :], hi6,
                    mybir.AluOpType.subtract)
                nc.vector.tensor_copy(xq[:, NHILO, :, :],
                                      xT[:, NHILO:NHILO + 2, :])
                nc.scalar.activation(
                    xq[:, NHILO + 1:NSLOT, :, :].rearrange(
                        "p a b f -> p (a b) f"),
                    xT[:, NHILO + 2:KO, :], COPY)
                return xq

            # ---- W: one resident read, |W| row-sums ---------------------
            wabs = scal.tile([P, NO], F32, name="wabs")
            w_tiles = {}
            for o2 in range(4):
                wt = wpool.tile([P, 2, K], F32, name=f"w_{o2}", tag="w")
                eng = nc.scalar if o2 < 2 else nc.sync
                eng.dma_start(
                    wt[:],
                    w_ap[o2 * 2 * P:(o2 + 1) * 2 * P, :].rearrange(
                        "(a p) k -> p a k", p=P))
                nc.vector.tensor_reduce(
                    wabs[:, 2 * o2:2 * o2 + 2], wt[:], mybir.AxisListType.X,
                    mybir.AluOpType.add, apply_absolute_value=True)
                w_tiles[o2] = wt

            # ---- global |W| AllReduce: triggered before the prefix ------
            wsum = scal.tile([P, 1], F32, name="wsum")
            nc.vector.tensor_reduce(
                wsum[:], wabs[:], mybir.AxisListType.X, mybir.AluOpType.add)
            tot128 = scal.tile([P, 1], F32, name="tot128")
            nc.gpsimd.partition_all_reduce(
                tot128[:], wsum[:], P, bass_isa.ReduceOp.add)

            cc_in = dram.tile([1, 1], F32, name="cc_in")
            cc_out = dram.tile([1, 1], F32, name="cc_out", addr_space="Shared")
            nc.gpsimd.dma_start(cc_in[:], tot128[0:1, :])
            nc.gpsimd.collective_compute(
                "AllReduce", mybir.AluOpType.add,
                replica_groups=[list(range(NCORES))],
                ins=[cc_in[:].opt()], outs=[cc_out[:].opt()])
            tot_sb = scal.tile([1, 1], F32, name="tot_sb")
            nc.gpsimd.dma_start(tot_sb[:], cc_out[:])
            bcast = scal.tile([P, 1], F32, name="bcast")
            nc.gpsimd.partition_broadcast(bcast[:], tot_sb[:])

            # ---- x prefix + dummy matmuls pacing the Tensor engine ------
            prefix_xq = {}
            xb_mt = [None]
            for mt in range(PREFIX):
                xt = x_load(mt)
                prefix_xq[mt] = x_stage(mt, xt, xb_out=xb_mt)
                for dj in range(DUM_PER_MT):
                    pd = psum_d.tile([P, 512], F32, name=f"pd_{mt}_{dj}",
                                     tag="pd")
                    nc.tensor.matmul(pd[:], lhsT=ident[:],
                                     rhs=xb_mt[0][:, 0:512],
                                     start=True, stop=True)
            for dj in range(DUM_TAIL):
                pd = psum_d.tile([P, 512], F32, name=f"pdt_{dj}", tag="pd")
                nc.tensor.matmul(pd[:], lhsT=ident[:],
                                 rhs=xb_mt[0][:, 0:512],
                                 start=True, stop=True)

            # ---- thresholds (after the prefix in the DVE FIFO) ----------
            thr_pos = scal.tile([P, 1], F32, name="thr_pos")
            nc.vector.tensor_scalar(
                thr_pos[:], bcast[:], 0.5 / W_ELEMS, 0.5e-4,
                mybir.AluOpType.mult, mybir.AluOpType.add)
            thr_neg = scal.tile([P, 1], F32, name="thr_neg")
            nc.vector.tensor_scalar(
                thr_neg[:], thr_pos[:], -1.0, None, mybir.AluOpType.mult)
            scale_col = scal.tile([P, 1], F32, name="scale_col")
            nc.vector.tensor_scalar(
                scale_col[:], bcast[:], 1.0 / W_ELEMS, 1e-4,
                mybir.AluOpType.mult, mybir.AluOpType.add)

            # ---- quantize + transpose -> qT [P, NSLOT, 2, NS] -----------
            qT = qT_pool.tile([P, NSLOT, 2, NS], FP8, name="qT")
            for o2 in range(4):
                wt = w_tiles[o2]
                qt = qtpool.tile([P, 2, K], FP8, name=f"qt_{o2}", tag="qt")
                nc.vector.tensor_scalar(
                    qt[:], wt[:], thr_pos[:], None, mybir.AluOpType.is_gt)
                gb = gpool.tile([P, 2, K], FP8, name=f"gb_{o2}", tag="gb")
                nc.vector.tensor_scalar(
                    gb[:], wt[:], thr_neg[:], None, mybir.AluOpType.is_lt)
                nc.vector.tensor_tensor(
                    qt[:], qt[:], gb[:], mybir.AluOpType.subtract)
                for h in range(2):
                    o = o2 * 2 + h
                    osl = slice(o * P, (o + 1) * P)
                    for g in range(4):
                        pt = psum_t.tile([P, 4, P], F32, name=f"ptq_{o}_{g}",
                                         tag="pt")
                        for j in range(4):
                            ko = g * 4 + j
                            nc.tensor.matmul(
                                pt[:, j, :],
                                lhsT=qt[:, h, ko * P:(ko + 1) * P],
                                rhs=identq[:], start=True, stop=True)
                        if g == 0:
                            nc.scalar.activation(
                                qT[:, 0:4, 0, osl], pt[:], COPY)
                            nc.scalar.activation(
                                qT[:, 0:4, 1, osl], pt[:], COPY)
                        elif g == 1:
                            nc.scalar.activation(
                                qT[:, 4:6, 0, osl], pt[:, 0:2, :], COPY)
                            nc.scalar.activation(
                                qT[:, 4:6, 1, osl], pt[:, 0:2, :], COPY)
                            nc.scalar.activation(
                                qT[:, 6, :, osl], pt[:, 2:4, :], COPY)
                        else:
                            j0 = NHILO + 1 + (g - 2) * 2
                            nc.scalar.activation(
                                qT[:, j0:j0 + 2, :, osl].rearrange(
                                    "p a b f -> p (a b) f"), pt[:], COPY)

            # ---- main loop: DoubleRow matmuls + scale + store -----------
            xq_ready = dict(prefix_xq)
            for mt in range(MT):
                tgt = mt + LOOKAHEAD
                if tgt < MT and tgt not in xq_ready:
                    xq_ready[tgt] = x_stage(tgt, x_load(tgt))
                if mt not in xq_ready:
                    xq_ready[mt] = x_stage(mt, x_load(mt))
                xq = xq_ready.pop(mt)
                ot = opool.tile([P, NS], F32, name=f"o_{mt}", tag="o")
                for nh in range(2):
                    po = psum_o.tile([P, 512], F32, name=f"po_{mt}_{nh}",
                                     tag="po")
                    for j in range(NSLOT):
                        nc.tensor.matmul(
                            po[:], lhsT=xq[:, j, :, :],
                            rhs=qT[:, j, :, nh * 512:(nh + 1) * 512],
                            start=(j == 0), stop=(j == NSLOT - 1),
                            perf_mode=DR)
                    nc.scalar.activation(
                        ot[:, nh * 512:(nh + 1) * 512], po[:],
                        COPY, scale=scale_col[:])
                nc.scalar.dma_start(o_ap[mt * P:(mt + 1) * P, :], ot[:])

    nc.compile()
    return nc


_NC_CACHE = None


def get_nc():
    global _NC_CACHE
    if _NC_CACHE is None:
        _NC_CACHE = build_nc()
    return _NC_CACHE


def make_in_maps(x, weight):
    x2 = np.ascontiguousarray(np.asarray(x, dtype=np.float32).reshape(M, K))
    w = np.asarray(weight, dtype=np.float32)
    return [
        {"x": x2, "w": np.ascontiguousarray(w[c * NS:(c + 1) * NS])}
        for c in range(NCORES)
    ]


def kernel(x, weight):
    nc = get_nc()
    in_maps = make_in_maps(x, weight)
    try:
        res = run_bass_kernel_spmd(nc, in_maps, list(range(NCORES)))
    except Exception:
        # transient device errors have been observed on first touch; retry once
        res = run_bass_kernel_spmd(nc, in_maps, list(range(NCORES)))
    out = np.concatenate(
        [res.results[c]["out"] for c in range(NCORES)], axis=1)
    return np.ascontiguousarray(out.reshape(4, 2048, N_FULL), dtype=np.float32)


# revision 13
# speedup vs baseline: 1.3944x; 1.3944x over previous
"""BitNet linear layer (b1.58-style) on 8 Trainium2 NeuronCores.

Computes: scale = 1e-4 + mean(|W|); q = clip(round(W/scale), -1, 1);
          out = scale * (x @ q.T)
for x [4, 2048, 2048] f32 and W [8192, 2048] f32.

Sharding: tensor-parallel over out_features. Each core gets the full x
(replicated) and a 1024-row shard of W; host concatenates the 8 per-core
[8192, 1024] outputs along the feature axis.

Per-core compute strategy (fp8 DoubleRow main loop):
  - The ternary q is exact in fp8e4. x is decomposed as
    hi = fp8(bf16(x)), lo = fp8(bf16(x) - hi). K-columns 0..NHILO*128-1
    are computed with (hi, lo) pairs (error ~bf16-level); the rest with
    hi only. NHILO=6 -> measured output rel err ~1.9e-2 < 2e-2 gate.
  - DoubleRow fp8 matmuls contract 256 SBUF rows per instruction at the
    same per-instruction cost as one 128-row bf16 matmul, so the main
    loop needs NSLOT=11 instructions per (m-tile, n-half) vs 16 bf16.
  - Layout: xq [128, NSLOT, 2, 128] per m-tile; j<NHILO -> (hi_j, lo_j),
    j>=NHILO -> (hi_a, hi_b) two k-tiles packed. qT mirrors it with q_j
    duplicated for hi/lo pairs.
  - x transposes are identity matmuls on the PE (~66ns per 128x128
    block when pipelined); quantization happens at PSUM drain time.
  - Startup: W + |W| AllReduce trigger first (with a junk warm-up
    AllReduce at t=0 to absorb CC ring setup); a shallow x prefix plus
    dummy identity matmuls keep the Tensor engine busy so HAM keeps the
    clock up; thresholds + q quantize follow the prefix in the DVE FIFO;
    the main loop stages x m-tiles LOOKAHEAD deep.
"""

import os
import sys

sys.path.insert(0, "/opt/trn_rl_repo")

import numpy as np

import concourse.bass as bass
import concourse.tile as tile
from concourse import bacc, mybir
from concourse.bass_utils import run_bass_kernel_spmd
from concourse.masks import make_identity
from concourse import bass_isa

F32 = mybir.dt.float32
BF16 = mybir.dt.bfloat16
FP8 = mybir.dt.float8e4

NCORES = 8
M = 8192          # tokens (4*2048)
K = 2048          # in_features
N_FULL = 8192     # out_features
NS = N_FULL // NCORES  # 1024 per-core shard
P = 128
KO = K // P       # 16 k-tiles
NO = NS // P      # 8 W-row tiles per shard
MT = M // P       # 64 m-tiles
W_ELEMS = float(N_FULL * K)  # 16777216, for the mean

NHILO = 6         # k-tiles 0..NHILO-1 get hi+lo pairs; rest hi only
NSLOT = NHILO + (KO - NHILO) // 2  # 11 DoubleRow slots
PREFIX = 10       # m-tiles of x pipeline emitted before quantize
LOOKAHEAD = 3     # m-tiles staged ahead of the main matmul loop
DUM_PER_MT = 13   # PE warm-up matmuls per prefix m-tile
DUM_TAIL = 160    # extra warm-up matmuls after the prefix

DR = mybir.MatmulPerfMode.DoubleRow
COPY = mybir.ActivationFunctionType.Copy


def build_nc():
    nc = bacc.Bacc("TRN2", target_bir_lowering=False, debug=False,
                   num_devices=NCORES)
    x_d = nc.dram_tensor("x", [M, K], F32, kind="ExternalInput")
    w_d = nc.dram_tensor("w", [NS, K], F32, kind="ExternalInput")
    o_d = nc.dram_tensor("out", [M, NS], F32, kind="ExternalOutput")
    x_ap, w_ap, o_ap = x_d.ap(), w_d.ap(), o_d.ap()

    with tile.TileContext(nc) as tc:
        with (
            tc.tile_pool(name="const", bufs=1) as const,
            tc.tile_pool(name="scal", bufs=1) as scal,
            tc.tile_pool(name="wpool", bufs=4) as wpool,
            tc.tile_pool(name="qtpool", bufs=2) as qtpool,
            tc.tile_pool(name="gpool", bufs=1) as gpool,
            tc.tile_pool(name="qT_pool", bufs=1) as qT_pool,
            tc.tile_pool(name="xpool", bufs=3) as xpool,
            tc.tile_pool(name="xbpool", bufs=3) as xbpool,
            tc.tile_pool(name="xqpool", bufs=PREFIX + LOOKAHEAD + 2) as xqpool,
            tc.tile_pool(name="opool", bufs=2) as opool,
            tc.tile_pool(name="psum_t", bufs=3, space="PSUM") as psum_t,
            tc.tile_pool(name="psum_o", bufs=3, space="PSUM") as psum_o,
            tc.tile_pool(name="psum_d", bufs=2, space="PSUM") as psum_d,
            tc.tile_pool(name="dram", bufs=1, space="DRAM") as dram,
        ):
            ident = const.tile([P, P], BF16, name="ident")
            make_identity(nc, ident)
            identq = const.tile([P, P], FP8, name="identq")
            make_identity(nc, identq)

            # ---- warm-up AllReduce on junk: absorbs CC ring setup -------
            ccw_in = dram.tile([1, 1], F32, name="ccw_in")
            ccw_out = dram.tile([1, 1], F32, name="ccw_out",
                                addr_space="Shared")
            nc.gpsimd.dma_start(ccw_in[:], ident[0:1, 0:1])
            nc.gpsimd.collective_compute(
                "AllReduce", mybir.AluOpType.add,
                replica_groups=[list(range(NCORES))],
                ins=[ccw_in[:].opt()], outs=[ccw_out[:].opt()])

            # ---- x pipeline stages --------------------------------------
            def x_load(mt):
                xt = xpool.tile([P, K], F32, name=f"x_{mt}", tag="x")
                nc.sync.dma_start(xt[:], x_ap[mt * P:(mt + 1) * P, :])
                return xt

            def x_stage(mt, xt, xb_out=None):
                """cast -> PE transpose -> quantize into xq slots.

                Slot layout (NHILO=6): psum group g holds transposed
                k-tiles 4g..4g+3.
                  g0: kos 0-3  -> hi xq[:,0:4,0,:], lo xq[:,0:4,1,:]
                  g1: kos 4,5  -> hi/lo xq[:,4:6,*,:]; kos 6,7 -> xq[:,6,:,:]
                  g2: kos 8-11 -> xq[:,7:9,:,:]   (ACT)
                  g3: kos 12-15-> xq[:,9:11,:,:]  (ACT)
                """
                xb = xbpool.tile([P, K], BF16, name=f"xb_{mt}", tag="xb")
                if xb_out is not None:
                    xb_out[0] = xb
                nc.vector.tensor_copy(xb[:], xt[:])
                xq = xqpool.tile([P, NSLOT, 2, P], FP8, name=f"xq_{mt}",
                                 tag="xq")
                for g in range(4):
                    pt = psum_t.tile([P, 4, P], F32, name=f"ptx_{mt}_{g}",
                                     tag="pt")
                    for j in range(4):
                        ko = g * 4 + j
                        nc.tensor.matmul(
                            pt[:, j, :],
                            lhsT=xb[:, ko * P:(ko + 1) * P],
                            rhs=ident[:], start=True, stop=True)
                    if g == 0:
                        hi = xq[:, 0:4, 0, :]
                        nc.vector.tensor_copy(hi, pt[:])
                        nc.vector.tensor_tensor(
                            xq[:, 0:4, 1, :], pt[:], hi,
                            mybir.AluOpType.subtract)
                    elif g == 1:
                        hi = xq[:, 4:6, 0, :]
                        nc.vector.tensor_copy(hi, pt[:, 0:2, :])
                        nc.vector.tensor_tensor(
                            xq[:, 4:6, 1, :], pt[:, 0:2, :], hi,
                            mybir.AluOpType.subtract)
                        nc.vector.tensor_copy(
                            xq[:, 6, :, :], pt[:, 2:4, :])
                    else:
                        j0 = NHILO + 1 + (g - 2) * 2
                        nc.scalar.activation(
                            xq[:, j0:j0 + 2, :, :].rearrange(
                                "p a b f -> p (a b) f"), pt[:], COPY)
                return xq

            # ---- W: one resident read, |W| row-sums ---------------------
            wabs = scal.tile([P, NO], F32, name="wabs")
            w_tiles = {}
            for o2 in range(4):
                wt = wpool.tile([P, 2, K], F32, name=f"w_{o2}", tag="w")
                eng = nc.sync if o2 < 2 else nc.scalar
                eng.dma_start(
                    wt[:],
                    w_ap[o2 * 2 * P:(o2 + 1) * 2 * P, :].rearrange(
                        "(a p) k -> p a k", p=P))
                nc.vector.tensor_reduce(
                    wabs[:, 2 * o2:2 * o2 + 2], wt[:], mybir.AxisListType.X,
                    mybir.AluOpType.add, apply_absolute_value=True)
                w_tiles[o2] = wt

            # ---- global |W| AllReduce: triggered before the prefix ------
            wsum = scal.tile([P, 1], F32, name="wsum")
            nc.vector.tensor_reduce(
                wsum[:], wabs[:], mybir.AxisListType.X, mybir.AluOpType.add)
            tot128 = scal.tile([P, 1], F32, name="tot128")
            nc.gpsimd.partition_all_reduce(
                tot128[:], wsum[:], P, bass_isa.ReduceOp.add)

            cc_in = dram.tile([1, 1], F32, name="cc_in")
            cc_out = dram.tile([1, 1], F32, name="cc_out", addr_space="Shared")
            nc.gpsimd.dma_start(cc_in[:], tot128[0:1, :])
            nc.gpsimd.collective_compute(
                "AllReduce", mybir.AluOpType.add,
                replica_groups=[list(range(NCORES))],
                ins=[cc_in[:].opt()], outs=[cc_out[:].opt()])
            tot_sb = scal.tile([1, 1], F32, name="tot_sb")
            nc.gpsimd.dma_start(tot_sb[:], cc_out[:])
            bcast = scal.tile([P, 1], F32, name="bcast")
            nc.gpsimd.partition_broadcast(bcast[:], tot_sb[:])

            # ---- x prefix + dummy matmuls pacing the Tensor engine ------
            prefix_xq = {}
            xb_mt = [None]
            for mt in range(PREFIX):
                xt = x_load(mt)
                prefix_xq[mt] = x_stage(mt, xt, xb_out=xb_mt)
                for dj in range(DUM_PER_MT):
                    pd = psum_d.tile([P, 512], F32, name=f"pd_{mt}_{dj}",
                                     tag="pd")
                    nc.tensor.matmul(pd[:], lhsT=ident[:],
                                     rhs=xb_mt[0][:, 0:512],
                                     start=True, stop=True)
            for dj in range(DUM_TAIL):
                pd = psum_d.tile([P, 512], F32, name=f"pdt_{dj}", tag="pd")
                nc.tensor.matmul(pd[:], lhsT=ident[:],
                                 rhs=xb_mt[0][:, 0:512],
                                 start=True, stop=True)

            # ---- thresholds (after the prefix in the DVE FIFO) ----------
            thr_pos = scal.tile([P, 1], F32, name="thr_pos")
            nc.vector.tensor_scalar(
                thr_pos[:], bcast[:], 0.5 / W_ELEMS, 0.5e-4,
                mybir.AluOpType.mult, mybir.AluOpType.add)
            thr_neg = scal.tile([P, 1], F32, name="thr_neg")
            nc.vector.tensor_scalar(
                thr_neg[:], thr_pos[:], -1.0, None, mybir.AluOpType.mult)
            scale_col = scal.tile([P, 1], F32, name="scale_col")
            nc.vector.tensor_scalar(
                scale_col[:], bcast[:], 1.0 / W_ELEMS, 1e-4,
                mybir.AluOpType.mult, mybir.AluOpType.add)

            # ---- quantize + transpose -> qT [P, NSLOT, 2, NS] -----------
            qT = qT_pool.tile([P, NSLOT, 2, NS], FP8, name="qT")
            for o2 in range(4):
                wt = w_tiles[o2]
                qt = qtpool.tile([P, 2, K], FP8, name=f"qt_{o2}", tag="qt")
                nc.vector.tensor_scalar(
                    qt[:], wt[:], thr_pos[:], None, mybir.AluOpType.is_gt)
                gb = gpool.tile([P, 2, K], FP8, name=f"gb_{o2}", tag="gb")
                nc.vector.tensor_scalar(
                    gb[:], wt[:], thr_neg[:], None, mybir.AluOpType.is_lt)
                nc.vector.tensor_tensor(
                    qt[:], qt[:], gb[:], mybir.AluOpType.subtract)
                for h in range(2):
                    o = o2 * 2 + h
                    osl = slice(o * P, (o + 1) * P)
                    for g in range(4):
                        pt = psum_t.tile([P, 4, P], F32, name=f"ptq_{o}_{g}",
                                         tag="pt")
                        for j in range(4):
                            ko = g * 4 + j
                            nc.tensor.matmul(
                                pt[:, j, :],
                                lhsT=qt[:, h, ko * P:(ko + 1) * P],
                                rhs=identq[:], start=True, stop=True)
                        if g == 0:
                            nc.scalar.activation(
                                qT[:, 0:4, 0, osl], pt[:], COPY)
                            nc.scalar.activation(
                                qT[:, 0:4, 1, osl], pt[:], COPY)
                        elif g == 1:
                            nc.scalar.activation(
                                qT[:, 4:6, 0, osl], pt[:, 0:2, :], COPY)
                            nc.scalar.activation(
                                qT[:, 4:6, 1, osl], pt[:, 0:2, :], COPY)
                            nc.scalar.activation(
                                qT[:, 6, :, osl], pt[:, 2:4, :], COPY)
                        else:
                            j0 = NHILO + 1 + (g - 2) * 2
                            nc.scalar.activation(
                                qT[:, j0:j0 + 2, :, osl].rearrange(
                                    "p a b f -> p (a b) f"), pt[:], COPY)

            # ---- main loop: DoubleRow matmuls + scale + store -----------
            xq_ready = dict(prefix_xq)
            for mt in range(MT):
                tgt = mt + LOOKAHEAD
                if tgt < MT and tgt not in xq_ready:
                    xq_ready[tgt] = x_stage(tgt, x_load(tgt))
                if mt not in xq_ready:
                    xq_ready[mt] = x_stage(mt, x_load(mt))
                xq = xq_ready.pop(mt)
                ot = opool.tile([P, NS], F32, name=f"o_{mt}", tag="o")
                for nh in range(2):
                    po = psum_o.tile([P, 512], F32, name=f"po_{mt}_{nh}",
                                     tag="po")
                    for j in range(NSLOT):
                        nc.tensor.matmul(
                            po[:], lhsT=xq[:, j, :, :],
                            rhs=qT[:, j, :, nh * 512:(nh + 1) * 512],
                            start=(j == 0), stop=(j == NSLOT - 1),
                            perf_mode=DR)
                    nc.scalar.activation(
                        ot[:, nh * 512:(nh + 1) * 512], po[:],
                        COPY, scale=scale_col[:])
                nc.scalar.dma_start(o_ap[mt * P:(mt + 1) * P, :], ot[:])

    nc.compile()
    return nc


_NC_CACHE = None


def get_nc():
    global _NC_CACHE
    if _NC_CACHE is None:
        _NC_CACHE = build_nc()
    return _NC_CACHE


def make_in_maps(x, weight):
    x2 = np.ascontiguousarray(np.asarray(x, dtype=np.float32).reshape(M, K))
    w = np.asarray(weight, dtype=np.float32)
    return [
        {"x": x2, "w": np.ascontiguousarray(w[c * NS:(c + 1) * NS])}
        for c in range(NCORES)
    ]


def kernel(x, weight):
    nc = get_nc()
    in_maps = make_in_maps(x, weight)
    try:
        res = run_bass_kernel_spmd(nc, in_maps, list(range(NCORES)))
    except Exception:
        # transient device errors have been observed on first touch; retry once
        res = run_bass_kernel_spmd(nc, in_maps, list(range(NCORES)))
    out = np.concatenate(
        [res.results[c]["out"] for c in range(NCORES)], axis=1)
    return np.ascontiguousarray(out.reshape(4, 2048, N_FULL), dtype=np.float32)


# revision 14
# speedup vs baseline: 1.4031x; 1.0062x over previous
"""BitNet linear layer (b1.58-style) on 8 Trainium2 NeuronCores.

Computes: scale = 1e-4 + mean(|W|); q = clip(round(W/scale), -1, 1);
          out = scale * (x @ q.T)
for x [4, 2048, 2048] f32 and W [8192, 2048] f32.

Sharding: tensor-parallel over out_features. Each core gets the full x
(replicated) and a 1024-row shard of W; host concatenates the 8 per-core
[8192, 1024] outputs along the feature axis.

Per-core compute strategy (fp8 DoubleRow main loop):
  - The ternary q is exact in fp8e4. x is decomposed as
    hi = fp8(bf16(x)), lo = fp8(bf16(x) - hi). K-columns 0..NHILO*128-1
    are computed with (hi, lo) pairs (error ~bf16-level); the rest with
    hi only. NHILO=6 -> measured output rel err ~1.9e-2 < 2e-2 gate.
  - DoubleRow fp8 matmuls contract 256 SBUF rows per instruction at the
    same per-instruction cost as one 128-row bf16 matmul, so the main
    loop needs NSLOT=11 instructions per (m-tile, n-half) vs 16 bf16.
  - Layout: xq [128, NSLOT, 2, 128] per m-tile; j<NHILO -> (hi_j, lo_j),
    j>=NHILO -> (hi_a, hi_b) two k-tiles packed. qT mirrors it with q_j
    duplicated for hi/lo pairs.
  - x transposes are identity matmuls on the PE (~66ns per 128x128
    block when pipelined); quantization happens at PSUM drain time.
  - Startup: W + |W| AllReduce trigger first (with a junk warm-up
    AllReduce at t=0 to absorb CC ring setup); a shallow x prefix plus
    dummy identity matmuls keep the Tensor engine busy so HAM keeps the
    clock up; thresholds + q quantize follow the prefix in the DVE FIFO;
    the main loop stages x m-tiles LOOKAHEAD deep.
"""

import os
import sys

sys.path.insert(0, "/opt/trn_rl_repo")

import numpy as np

import concourse.bass as bass
import concourse.tile as tile
from concourse import bacc, mybir
from concourse.bass_utils import run_bass_kernel_spmd
from concourse.masks import make_identity
from concourse import bass_isa

F32 = mybir.dt.float32
BF16 = mybir.dt.bfloat16
FP8 = mybir.dt.float8e4

NCORES = 8
M = 8192          # tokens (4*2048)
K = 2048          # in_features
N_FULL = 8192     # out_features
NS = N_FULL // NCORES  # 1024 per-core shard
P = 128
KO = K // P       # 16 k-tiles
NO = NS // P      # 8 W-row tiles per shard
MT = M // P       # 64 m-tiles
W_ELEMS = float(N_FULL * K)  # 16777216, for the mean

NHILO = 6         # k-tiles 0..NHILO-1 get hi+lo pairs; rest hi only
NSLOT = NHILO + (KO - NHILO) // 2  # 11 DoubleRow slots
PREFIX = 12       # m-tiles of x pipeline emitted before quantize
LOOKAHEAD = 3     # m-tiles staged ahead of the main matmul loop
DUM_PER_MT = 13   # PE warm-up matmuls per prefix m-tile
DUM_TAIL = 120    # extra warm-up matmuls after the prefix
DUM_Q = 14        # warm-up matmuls per W pair-tile during quantize

DR = mybir.MatmulPerfMode.DoubleRow
COPY = mybir.ActivationFunctionType.Copy


def build_nc():
    nc = bacc.Bacc("TRN2", target_bir_lowering=False, debug=False,
                   num_devices=NCORES)
    x_d = nc.dram_tensor("x", [M, K], F32, kind="ExternalInput")
    w_d = nc.dram_tensor("w", [NS, K], F32, kind="ExternalInput")
    o_d = nc.dram_tensor("out", [M, NS], F32, kind="ExternalOutput")
    x_ap, w_ap, o_ap = x_d.ap(), w_d.ap(), o_d.ap()

    with tile.TileContext(nc) as tc:
        with (
            tc.tile_pool(name="const", bufs=1) as const,
            tc.tile_pool(name="scal", bufs=1) as scal,
            tc.tile_pool(name="wpool", bufs=4) as wpool,
            tc.tile_pool(name="qtpool", bufs=2) as qtpool,
            tc.tile_pool(name="gpool", bufs=1) as gpool,
            tc.tile_pool(name="qT_pool", bufs=1) as qT_pool,
            tc.tile_pool(name="xpool", bufs=3) as xpool,
            tc.tile_pool(name="xbpool", bufs=3) as xbpool,
            tc.tile_pool(name="xqpool", bufs=PREFIX + LOOKAHEAD + 2) as xqpool,
            tc.tile_pool(name="opool", bufs=2) as opool,
            tc.tile_pool(name="psum_t", bufs=3, space="PSUM") as psum_t,
            tc.tile_pool(name="psum_o", bufs=3, space="PSUM") as psum_o,
            tc.tile_pool(name="psum_d", bufs=2, space="PSUM") as psum_d,
            tc.tile_pool(name="dram", bufs=1, space="DRAM") as dram,
        ):
            ident = const.tile([P, P], BF16, name="ident")
            make_identity(nc, ident)
            identq = const.tile([P, P], FP8, name="identq")
            make_identity(nc, identq)

            # ---- warm-up AllReduce on junk: absorbs CC ring setup -------
            ccw_in = dram.tile([1, 1], F32, name="ccw_in")
            ccw_out = dram.tile([1, 1], F32, name="ccw_out",
                                addr_space="Shared")
            nc.gpsimd.dma_start(ccw_in[:], ident[0:1, 0:1])
            nc.gpsimd.collective_compute(
                "AllReduce", mybir.AluOpType.add,
                replica_groups=[list(range(NCORES))],
                ins=[ccw_in[:].opt()], outs=[ccw_out[:].opt()])

            # ---- x pipeline stages --------------------------------------
            def x_load(mt):
                xt = xpool.tile([P, K], F32, name=f"x_{mt}", tag="x")
                nc.sync.dma_start(xt[:], x_ap[mt * P:(mt + 1) * P, :])
                return xt

            def x_stage(mt, xt, xb_out=None, cast_on_act=False):
                """cast -> PE transpose -> quantize into xq slots.

                Slot layout (NHILO=6): psum group g holds transposed
                k-tiles 4g..4g+3.
                  g0: kos 0-3  -> hi xq[:,0:4,0,:], lo xq[:,0:4,1,:]
                  g1: kos 4,5  -> hi/lo xq[:,4:6,*,:]; kos 6,7 -> xq[:,6,:,:]
                  g2: kos 8-11 -> xq[:,7:9,:,:]   (ACT)
                  g3: kos 12-15-> xq[:,9:11,:,:]  (ACT)
                """
                xb = xbpool.tile([P, K], BF16, name=f"xb_{mt}", tag="xb")
                if xb_out is not None:
                    xb_out[0] = xb
                if cast_on_act:
                    nc.scalar.activation(xb[:], xt[:], COPY)
                else:
                    nc.vector.tensor_copy(xb[:], xt[:])
                xq = xqpool.tile([P, NSLOT, 2, P], FP8, name=f"xq_{mt}",
                                 tag="xq")
                for g in range(4):
                    pt = psum_t.tile([P, 4, P], F32, name=f"ptx_{mt}_{g}",
                                     tag="pt")
                    for j in range(4):
                        ko = g * 4 + j
                        nc.tensor.matmul(
                            pt[:, j, :],
                            lhsT=xb[:, ko * P:(ko + 1) * P],
                            rhs=ident[:], start=True, stop=True)
                    if g == 0:
                        hi = xq[:, 0:4, 0, :]
                        nc.vector.tensor_copy(hi, pt[:])
                        nc.vector.tensor_tensor(
                            xq[:, 0:4, 1, :], pt[:], hi,
                            mybir.AluOpType.subtract)
                    elif g == 1:
                        hi = xq[:, 4:6, 0, :]
                        nc.vector.tensor_copy(hi, pt[:, 0:2, :])
                        nc.vector.tensor_tensor(
                            xq[:, 4:6, 1, :], pt[:, 0:2, :], hi,
                            mybir.AluOpType.subtract)
                        nc.vector.tensor_copy(
                            xq[:, 6, :, :], pt[:, 2:4, :])
                    else:
                        j0 = NHILO + 1 + (g - 2) * 2
                        nc.scalar.activation(
                            xq[:, j0:j0 + 2, :, :].rearrange(
                                "p a b f -> p (a b) f"), pt[:], COPY)
                return xq

            # ---- W: one resident read, |W| row-sums ---------------------
            wabs = scal.tile([P, NO], F32, name="wabs")
            w_tiles = {}
            for o2 in range(4):
                wt = wpool.tile([P, 2, K], F32, name=f"w_{o2}", tag="w")
                for h in range(2):
                    o = 2 * o2 + h
                    eng = nc.sync if o % 2 == 0 else nc.scalar
                    eng.dma_start(
                        wt[:, h, :], w_ap[o * P:(o + 1) * P, :])
                    nc.vector.tensor_reduce(
                        wabs[:, o:o + 1], wt[:, h, :], mybir.AxisListType.X,
                        mybir.AluOpType.add, apply_absolute_value=True)
                w_tiles[o2] = wt

            # ---- global |W| AllReduce: triggered before the prefix ------
            wsum = scal.tile([P, 1], F32, name="wsum")
            nc.vector.tensor_reduce(
                wsum[:], wabs[:], mybir.AxisListType.X, mybir.AluOpType.add)
            tot128 = scal.tile([P, 1], F32, name="tot128")
            nc.gpsimd.partition_all_reduce(
                tot128[:], wsum[:], P, bass_isa.ReduceOp.add)

            cc_in = dram.tile([1, 1], F32, name="cc_in")
            cc_out = dram.tile([1, 1], F32, name="cc_out", addr_space="Shared")
            nc.gpsimd.dma_start(cc_in[:], tot128[0:1, :])
            nc.gpsimd.collective_compute(
                "AllReduce", mybir.AluOpType.add,
                replica_groups=[list(range(NCORES))],
                ins=[cc_in[:].opt()], outs=[cc_out[:].opt()])
            tot_sb = scal.tile([1, 1], F32, name="tot_sb")
            nc.gpsimd.dma_start(tot_sb[:], cc_out[:])
            bcast = scal.tile([P, 1], F32, name="bcast")
            nc.gpsimd.partition_broadcast(bcast[:], tot_sb[:])

            # ---- x prefix + dummy matmuls pacing the Tensor engine ------
            prefix_xq = {}
            xb_mt = [None]
            for mt in range(PREFIX):
                xt = x_load(mt)
                prefix_xq[mt] = x_stage(mt, xt, xb_out=xb_mt,
                                        cast_on_act=True)
                for dj in range(DUM_PER_MT):
                    pd = psum_d.tile([P, 512], F32, name=f"pd_{mt}_{dj}",
                                     tag="pd")
                    nc.tensor.matmul(pd[:], lhsT=ident[:],
                                     rhs=xb_mt[0][:, 0:512],
                                     start=True, stop=True)
            for dj in range(DUM_TAIL):
                pd = psum_d.tile([P, 512], F32, name=f"pdt_{dj}", tag="pd")
                nc.tensor.matmul(pd[:], lhsT=ident[:],
                                 rhs=xb_mt[0][:, 0:512],
                                 start=True, stop=True)

            # ---- thresholds (after the prefix in the DVE FIFO) ----------
            thr_pos = scal.tile([P, 1], F32, name="thr_pos")
            nc.vector.tensor_scalar(
                thr_pos[:], bcast[:], 0.5 / W_ELEMS, 0.5e-4,
                mybir.AluOpType.mult, mybir.AluOpType.add)
            thr_neg = scal.tile([P, 1], F32, name="thr_neg")
            nc.vector.tensor_scalar(
                thr_neg[:], thr_pos[:], -1.0, None, mybir.AluOpType.mult)
            scale_col = scal.tile([P, 1], F32, name="scale_col")
            nc.vector.tensor_scalar(
                scale_col[:], bcast[:], 1.0 / W_ELEMS, 1e-4,
                mybir.AluOpType.mult, mybir.AluOpType.add)

            # ---- quantize + transpose -> qT [P, NSLOT, 2, NS] -----------
            qT = qT_pool.tile([P, NSLOT, 2, NS], FP8, name="qT")
            for o2 in range(4):
                wt = w_tiles[o2]
                qt = qtpool.tile([P, 2, K], FP8, name=f"qt_{o2}", tag="qt")
                nc.vector.tensor_scalar(
                    qt[:], wt[:], thr_pos[:], None, mybir.AluOpType.is_gt)
                gb = gpool.tile([P, 2, K], FP8, name=f"gb_{o2}", tag="gb")
                nc.vector.tensor_scalar(
                    gb[:], wt[:], thr_neg[:], None, mybir.AluOpType.is_lt)
                nc.vector.tensor_tensor(
                    qt[:], qt[:], gb[:], mybir.AluOpType.subtract)
                for dj in range(DUM_Q):
                    pd = psum_d.tile([P, 512], F32, name=f"pdq_{o2}_{dj}",
                                     tag="pd")
                    nc.tensor.matmul(pd[:], lhsT=ident[:],
                                     rhs=xb_mt[0][:, 0:512],
                                     start=True, stop=True)
                for h in range(2):
                    o = o2 * 2 + h
                    osl = slice(o * P, (o + 1) * P)
                    for g in range(4):
                        pt = psum_t.tile([P, 4, P], F32, name=f"ptq_{o}_{g}",
                                         tag="pt")
                        for j in range(4):
                            ko = g * 4 + j
                            nc.tensor.matmul(
                                pt[:, j, :],
                                lhsT=qt[:, h, ko * P:(ko + 1) * P],
                                rhs=identq[:], start=True, stop=True)
                        if g == 0:
                            nc.scalar.activation(
                                qT[:, 0:4, 0, osl], pt[:], COPY)
                            nc.scalar.activation(
                                qT[:, 0:4, 1, osl], pt[:], COPY)
                        elif g == 1:
                            nc.scalar.activation(
                                qT[:, 4:6, 0, osl], pt[:, 0:2, :], COPY)
                            nc.scalar.activation(
                                qT[:, 4:6, 1, osl], pt[:, 0:2, :], COPY)
                            nc.scalar.activation(
                                qT[:, 6, :, osl], pt[:, 2:4, :], COPY)
                        else:
                            j0 = NHILO + 1 + (g - 2) * 2
                            nc.scalar.activation(
                                qT[:, j0:j0 + 2, :, osl].rearrange(
                                    "p a b f -> p (a b) f"), pt[:], COPY)

            # ---- main loop: DoubleRow matmuls + scale + store -----------
            xq_ready = dict(prefix_xq)
            for mt in range(MT):
                tgt = mt + LOOKAHEAD
                if tgt < MT and tgt not in xq_ready:
                    xq_ready[tgt] = x_stage(tgt, x_load(tgt))
                if mt not in xq_ready:
                    xq_ready[mt] = x_stage(mt, x_load(mt))
                xq = xq_ready.pop(mt)
                ot = opool.tile([P, NS], F32, name=f"o_{mt}", tag="o")
                for nh in range(2):
                    po = psum_o.tile([P, 512], F32, name=f"po_{mt}_{nh}",
                                     tag="po")
                    for j in range(NSLOT):
                        nc.tensor.matmul(
                            po[:], lhsT=xq[:, j, :, :],
                            rhs=qT[:, j, :, nh * 512:(nh + 1) * 512],
                            start=(j == 0), stop=(j == NSLOT - 1),
                            perf_mode=DR)
                    nc.scalar.activation(
                        ot[:, nh * 512:(nh + 1) * 512], po[:],
                        COPY, scale=scale_col[:])
                nc.scalar.dma_start(o_ap[mt * P:(mt + 1) * P, :], ot[:])

    nc.compile()
    return nc


_NC_CACHE = None


def get_nc():
    global _NC_CACHE
    if _NC_CACHE is None:
        _NC_CACHE = build_nc()
    return _NC_CACHE


def make_in_maps(x, weight):
    x2 = np.ascontiguousarray(np.asarray(x, dtype=np.float32).reshape(M, K))
    w = np.asarray(weight, dtype=np.float32)
    return [
        {"x": x2, "w": np.ascontiguousarray(w[c * NS:(c + 1) * NS])}
        for c in range(NCORES)
    ]


def kernel(x, weight):
    nc = get_nc()
    in_maps = make_in_maps(x, weight)
    try:
        res = run_bass_kernel_spmd(nc, in_maps, list(range(NCORES)))
    except Exception:
        # transient device errors have been observed on first touch; retry once
        res = run_bass_kernel_spmd(nc, in_maps, list(range(NCORES)))
    out = np.concatenate(
        [res.results[c]["out"] for c in range(NCORES)], axis=1)
    return np.ascontiguousarray(out.reshape(4, 2048, N_FULL), dtype=np.float32)
